# revision 4
# baseline (speedup 1.0000x reference)
"""Causal multi-head attention (B=4, S=1024, D=1024, H=16, hd=64) on 8 TRN2 cores.

Sharding: head-parallel. Core c owns heads {2c, 2c+1} for all batches, i.e.
d-columns [128c, 128c+128) of q/k/v/out. Each core runs independent causal
attention for its 8 (batch, head) pairs; no collectives.

Per-core algorithm (per head):
  - load q/k (2 heads packed, [128, 1024] SBUF tiles), transpose to qT/kT
    [hd, S] layouts via PE matmuls against an identity (head pair stacked on
    partitions 0:64 / 64:128).
  - scoresT[kc, qr] = kT.T @ qT via PE, blocks [128kc x <=512qr], causally
    skipping fully-masked blocks; partial diagonal blocks only compute the
    qr >= kc_block_start region and get a [128,128] lower-triangular -1e30
    mask added before exp.
  - exp(scale * scoresT) on ACT (no max subtraction: q,k ~ N(0,1) => scores
    are O(6), exp stays in fp32 range).
  - out[qr, hd] and the softmax denominator come from one PE accumulation:
    lhsT = expT block slices [kc, qr], rhs = v_aug [kc, 65] (v plus a ones
    column, prepared host-side), accumulated over kc chunks into PSUM.
  - normalize with per-row reciprocal of column 64, write packed out tile,
    DMA out.

The identity, the causal mask tile and the ones-augmented v are prepared on
the host and passed as extra ExternalInputs so that no on-chip instruction
depends on more than two producer engines (TRN2 instructions have a very
small number of semaphore wait slots).
"""

import sys

sys.path.insert(0, "/opt/trn_rl_repo")

import numpy as np

import concourse.bass as bass
import concourse.mybir as mybir
import concourse.tile as tile
from concourse import bass_utils

B, S, D, H = 4, 1024, 1024, 16
HD = 64
NCORES = 8
HPC = H // NCORES          # heads per core = 2
CW = HPC * HD              # per-core d-column width = 128
P = 128                    # partitions
NT = S // P                # 8 s-tiles of 128
QCH = 512                  # qr chunk width
NQC = S // QCH             # 2 qr chunks per head
SCALE = HD ** -0.5
FP32 = mybir.dt.float32
NEG = -1.0e30


def split_multi_waits(nc):
    """TRN2 TPB instructions carry exactly one semaphore wait slot; walrus
    refuses >1 on_wait per instruction.  Hoist extra waits onto standalone
    EventSemaphore instructions on the same engine, inserted right before the
    owning instruction (engines dispatch in order, so semantics are
    unchanged)."""
    ctr = [0]
    for fn in nc.m.functions:
        for blk in fn.blocks:
            insts = list(blk.instructions)
            out = []
            for inst in insts:
                si = inst.sync_info
                if si is not None and len(si.on_wait) > 1:
                    waits = list(si.on_wait)
                    for w in waits[:-1]:
                        ev = mybir.InstEventSemaphore(
                            name=f"evw-split-{ctr[0]}", ins=[], outs=[]
                        )
                        ctr[0] += 1
                        ev.engine = inst.engine
                        ev.sync_info = mybir.SyncInfo(on_wait=[w], on_update=[])
                        out.append(ev)
                    inst.sync_info = mybir.SyncInfo(
                        on_wait=[waits[-1]], on_update=list(si.on_update)
                    )
                out.append(inst)
            if len(out) != len(insts):
                for i, inst in enumerate(out):
                    existing = blk.instructions
                    if i < len(existing) and existing[i].name == inst.name:
                        continue
                    blk.instructions.insert(i, inst)


def _blocks_for_chunk(c):
    """scoresT blocks for qr chunk c: list of (j, r, W, qr_start).

    j = kc chunk index (kc in [128j, 128j+128)); only j <= last q-tile of the
    chunk survives causality. r>0 marks blocks whose qr range is narrowed to
    [c*512 + 128r, (c+1)*512); the first 128 columns of every partial block
    (j >= 4c) get the triangular mask.
    """
    out = []
    for j in range(4 * c + 4):
        r = max(0, j - 4 * c)
        w = QCH - P * r
        out.append((j, r, w, c * QCH + P * r))
    return out


def build_program():
    nc = bass.Bass(trn_type="TRN2")
    q_d = nc.dram_tensor("query", [B, S, CW], FP32, kind="ExternalInput")
    k_d = nc.dram_tensor("key", [B, S, CW], FP32, kind="ExternalInput")
    va_d = nc.dram_tensor("value_aug", [B, HPC, S, HD + 1], FP32,
                          kind="ExternalInput")
    id_d = nc.dram_tensor("ident_in", [P, P], FP32, kind="ExternalInput")
    mk_d = nc.dram_tensor("mask_in", [P, P], FP32, kind="ExternalInput")
    o_d = nc.dram_tensor("attn_out", [B, S, CW], FP32, kind="ExternalOutput")

    # per-head expT layout: offsets of each (c, j) block in a [P, 4608] tile
    exp_off = {}
    off = 0
    for c in range(NQC):
        for (j, r, w, qs) in _blocks_for_chunk(c):
            exp_off[(c, j)] = off
            off += w
    exp_cols = off  # 4608

    with tile.TileContext(nc) as tc:
        with (
            tc.tile_pool(name="const", bufs=1) as constp,
            tc.tile_pool(name="io", bufs=2) as iop,
            tc.tile_pool(name="outp", bufs=2) as outp,
            tc.tile_pool(name="trp", bufs=2) as trp,
            tc.tile_pool(name="vaugp", bufs=3) as vaugp,
            tc.tile_pool(name="expp", bufs=2) as expp,
            tc.tile_pool(name="smallp", bufs=8) as smallp,
            tc.tile_pool(name="ptr", bufs=2, space="PSUM") as ptr,
            tc.tile_pool(name="psc", bufs=3, space="PSUM") as psc,
            tc.tile_pool(name="pout", bufs=3, space="PSUM") as pout,
        ):
            ident = constp.tile([P, P], FP32)
            nc.sync.dma_start(ident, id_d[:])
            mask = constp.tile([P, P], FP32)
            nc.sync.dma_start(mask, mk_d[:])

            for b in range(B):
                # packed loads: col t*128 + jj  <->  dram[t*128 + p, jj]
                q_sb = iop.tile([P, S], FP32, tag="q_sb")
                k_sb = iop.tile([P, S], FP32, tag="k_sb")
                nc.sync.dma_start(
                    q_sb.rearrange("p (t j) -> p t j", j=CW),
                    q_d[b].rearrange("(t p) j -> p t j", p=P),
                )
                nc.sync.dma_start(
                    k_sb.rearrange("p (t j) -> p t j", j=CW),
                    k_d[b].rearrange("(t p) j -> p t j", p=P),
                )

                # transpose to qT/kT [128, S]: partition p<64 -> head0 d=p,
                # p>=64 -> head1 d=p-64; column = s.  out = src_slice.T @ I.
                qT = trp.tile([P, S], FP32, tag="qT")
                kT = trp.tile([P, S], FP32, tag="kT")
                for src, dst in ((q_sb, qT), (k_sb, kT)):
                    for half in range(2):
                        pt_t = ptr.tile([P, 512], FP32, tag="pt")
                        for tt in range(4):
                            t = half * 4 + tt
                            nc.tensor.matmul(
                                pt_t[:, tt * P:(tt + 1) * P],
                                src[:, t * P:(t + 1) * P],
                                ident,
                                start=True,
                                stop=True,
                            )
                        nc.vector.tensor_copy(
                            dst[:, half * 512:(half + 1) * 512], pt_t[:]
                        )

                out_sb = outp.tile([P, S], FP32, tag="out_sb")

                for hl in range(HPC):
                    pbase = hl * HD  # partition base for this head in qT/kT
                    # v with ones column: [128, 8*65]; block j at cols 65j.
                    v_aug = vaugp.tile([P, NT * (HD + 1)], FP32, tag="v_aug")
                    v_view = v_aug.rearrange("p (j e) -> p j e", e=HD + 1)
                    nc.sync.dma_start(
                        v_view,
                        va_d[b, hl].rearrange("(j p) e -> p j e", p=P),
                    )

                    expT = expp.tile([P, exp_cols], FP32, tag="expT")
                    for c in range(NQC):
                        for (j, r, w, qs) in _blocks_for_chunk(c):
                            ps_t = psc.tile([P, 512], FP32, tag="ps")
                            nc.tensor.matmul(
                                ps_t[:, 0:w],
                                kT[pbase:pbase + HD, j * P:(j + 1) * P],
                                qT[pbase:pbase + HD, qs:qs + w],
                                start=True,
                                stop=True,
                            )
                            if j >= 4 * c:
                                nc.vector.tensor_add(
                                    ps_t[:, 0:P], ps_t[:, 0:P], mask
                                )
                            o0 = exp_off[(c, j)]
                            nc.scalar.activation(
                                expT[:, o0:o0 + w],
                                ps_t[:, 0:w],
                                mybir.ActivationFunctionType.Exp,
                                scale=SCALE,
                            )

                    for qt in range(NT):
                        c = qt // 4
                        po_t = pout.tile([P, HD + 1], FP32, tag="po")
                        for j in range(qt + 1):
                            r = max(0, j - 4 * c)
                            qs = c * QCH + P * r
                            o0 = exp_off[(c, j)] + qt * P - qs
                            nc.tensor.matmul(
                                po_t[:],
                                expT[:, o0:o0 + P],
                                v_view[:, j, :],
                                start=(j == 0),
                                stop=(j == qt),
                            )
                        recip = smallp.tile([P, 1], FP32, tag="recip")
                        nc.vector.reciprocal(recip, po_t[:, HD:HD + 1])
                        nc.vector.tensor_scalar_mul(
                            out_sb[:, qt * P + hl * HD: qt * P + (hl + 1) * HD],
                            po_t[:, 0:HD],
                            recip,
                        )

                nc.sync.dma_start(
                    o_d[b].rearrange("(t p) j -> p t j", p=P),
                    out_sb.rearrange("p (t j) -> p t j", j=CW),
                )
    split_multi_waits(nc)
    return nc


_PROGRAM = None

_IDENT = np.eye(P, dtype=np.float32)
_MASK = np.where(
    np.arange(P)[None, :] >= np.arange(P)[:, None], 0.0, NEG
).astype(np.float32)


def kernel(query: np.ndarray, key: np.ndarray, value: np.ndarray) -> np.ndarray:
    global _PROGRAM
    if _PROGRAM is None:
        _PROGRAM = build_program()
    nc = _PROGRAM

    query = np.asarray(query, dtype=np.float32)
    key = np.asarray(key, dtype=np.float32)
    value = np.asarray(value, dtype=np.float32)

    in_maps = []
    for c in range(NCORES):
        sl = slice(c * CW, (c + 1) * CW)
        v_shard = value[:, :, sl]  # [B, S, CW]
        v_aug = np.ones((B, HPC, S, HD + 1), dtype=np.float32)
        for hl in range(HPC):
            v_aug[:, hl, :, :HD] = v_shard[:, :, hl * HD:(hl + 1) * HD]
        in_maps.append(
            {
                "query": np.ascontiguousarray(query[:, :, sl]),
                "key": np.ascontiguousarray(key[:, :, sl]),
                "value_aug": v_aug,
                "ident_in": _IDENT,
                "mask_in": _MASK,
            }
        )

    res = bass_utils.run_bass_kernel_spmd(nc, in_maps, core_ids=list(range(NCORES)))
    shards = [res.results[c]["attn_out"] for c in range(NCORES)]
    return np.concatenate(shards, axis=2)
